# revision 20
# baseline (speedup 1.0000x reference)
import numpy as np
import concourse.bass as bass
import concourse.tile as tile
import concourse.mybir as mybir
from concourse.bass_utils import run_bass_kernel_spmd
from concourse.vector_clock import ScopedClock

AF = mybir.ActivationFunctionType
f32 = mybir.dt.float32
f32r = mybir.dt.float32r


# ---- walrus workaround: at most 1 sync wait per engine instruction ----
def _patched_drain_and_barrier(self, tick_clock, wait_clock):
    drain_inst = self.nc.sync.drain()
    wait_clock.add_sem_waits(
        drain_inst.ins, ScopedClock({None: tick_clock.global_clock})
    )
    si = drain_inst.ins.sync_info
    if si is not None and len(si.on_wait) > 1:
        waits = list(si.on_wait)
        drain_inst.ins.sync_info = mybir.SyncInfo(
            on_wait=waits[:1], on_update=list(si.on_update)
        )
        for w in waits[1:]:
            nop = self.nc.sync.nop(nofuse=True)
            nop.ins.sync_info = mybir.SyncInfo(on_wait=[w], on_update=[])
    self.nc.all_engine_barrier()
    assert self.sems is not None
    popped = self.nc._tile_sem_poison_stack.pop()
    assert popped is self._sem_poison
    self.nc.clear_and_free_semaphores(list(self.sems.allocated().values()))
    self.nc.all_engine_barrier()


tile.TileContext._drain_and_barrier = _patched_drain_and_barrier

_orig_commit = tile.TileContext._commit_instruction


def _commit_split_waits(self, inst, lazy_reg_writes=True):
    si = inst.sync_info
    if (
        si is not None
        and len(si.on_wait) > 1
        and inst.engine != mybir.EngineType.Unassigned
    ):
        waits = list(si.on_wait)
        for w in waits[:-1]:
            nop = mybir.InstNoOp(
                name=self.nc.get_next_instruction_name(),
                sync_info=mybir.SyncInfo(on_wait=[w], on_update=[]),
                bass_nofuse=True,
                engine=inst.engine,
            )
            _orig_commit(self, nop, lazy_reg_writes=False)
        inst.sync_info = mybir.SyncInfo(
            on_wait=[waits[-1]], on_update=list(si.on_update)
        )
    _orig_commit(self, inst, lazy_reg_writes)


tile.TileContext._commit_instruction = _commit_split_waits


B, S, E, H, Dh = 4, 8192, 768, 12, 64
N_CORES = 8
SC = S // 2      # tokens per core
NT = SC // 128   # 32 output tiles
NW = SC // 512   # 8 query windows
KC = E // 128    # 6 contraction chunks


def _build_program():
    nc = bass.Bass()
    xk_t = nc.declare_dram_parameter("xk_t", [NT, 128, KC, 128], f32, isOutput=False)
    xv_t = nc.declare_dram_parameter("xv_t", [NT, 128, KC, 128], f32, isOutput=False)
    xq_t = nc.declare_dram_parameter("xq_t", [NW, 128, KC, 512], f32, isOutput=False)
    wk_h = nc.declare_dram_parameter("wk_h", [128, KC, E], f32, isOutput=False)
    wv_h = nc.declare_dram_parameter("wv_h", [128, KC, E], f32, isOutput=False)
    wq_h = nc.declare_dram_parameter("wq_h", [128, KC, E], f32, isOutput=False)
    wo_h = nc.declare_dram_parameter("wo_h", [128, KC, E], f32, isOutput=False)
    bkb_h = nc.declare_dram_parameter("bkb_h", [128, E], f32, isOutput=False)
    bvb_h = nc.declare_dram_parameter("bvb_h", [128, E], f32, isOutput=False)
    bqc_h = nc.declare_dram_parameter("bqc_h", [128, KC], f32, isOutput=False)
    boc_h = nc.declare_dram_parameter("boc_h", [128, KC], f32, isOutput=False)
    y_t = nc.declare_dram_parameter("y_t", [NW, 128, KC, 512], f32, isOutput=True)

    with tile.TileContext(nc) as tc:
        with (
            tc.tile_pool(name="main", bufs=1) as main,
            tc.tile_pool(name="dram", bufs=1, space="DRAM") as dram,
        ):
            wq_sb = main.tile([128, KC, E], f32r, tag="wq")
            wo_sb = main.tile([128, KC, E], f32r, tag="wo")
            bkb_sb = main.tile([128, E], f32, tag="bkb")
            bvb_sb = main.tile([128, E], f32, tag="bvb")
            bqc_sb = main.tile([128, KC], f32, tag="bqc")
            boc_sb = main.tile([128, KC], f32, tag="boc")

            kvbd = main.tile([128, KC, 128], f32r, tag="kvbd")
            kv_in = dram.tile([128, KC, 128], f32, tag="kvin")
            kv_out = dram.tile([128, KC, 128], f32, tag="kvout")

            # spread weight loads over per-engine DMA queues so they land
            # in parallel with each other and with the x-tile DMAs (sync q)
            nc.gpsimd.dma_start(bkb_sb[:], bkb_h[:])
            nc.gpsimd.dma_start(bvb_sb[:], bvb_h[:])
            nc.gpsimd.dma_start(bqc_sb[:], bqc_h[:])
            nc.gpsimd.dma_start(boc_sb[:], boc_h[:])

            # query windows live in the persistent pool so their DMAs (gpsimd
            # queue) overlap phase 1; loaded 0..NW-1 in order, bufs rotate
            xqs = {}

            def load_xq(n):
                t = main.tile([128, KC, 512], f32r, tag="xq", bufs=3,
                              name=f"xq{n}")
                nc.gpsimd.dma_start(out=t[:], in_=xq_t[n, :, :, :].bitcast(f32r))
                xqs[n] = t

            # ---------------- phase 1: K/V projection + KV accumulation ----
            with (
                tc.tile_pool(name="p1", bufs=1) as p1,
                tc.tile_pool(name="pp1", bufs=1, space="PSUM") as pp1,
            ):
                kvc = p1.tile([128, KC, 128], f32, tag="kvc")
                wk_sb = p1.tile([128, KC, E], f32r, tag="wk")
                wv_sb = p1.tile([128, KC, E], f32r, tag="wv")
                nc.scalar.dma_start(wk_sb[:], wk_h[:].bitcast(f32r))
                nc.gpsimd.dma_start(wv_sb[:], wv_h[:].bitcast(f32r))
                nc.scalar.dma_start(wq_sb[:], wq_h[:].bitcast(f32r))
                nc.scalar.dma_start(wo_sb[:], wo_h[:].bitcast(f32r))
                for n in range(3):
                    load_xq(n)

                kvp = [
                    pp1.tile([128, 128], f32, tag=f"kv{c}", name=f"kvp{c}")
                    for c in range(KC)
                ]
                for t in range(NT):
                    xk_sb = p1.tile([128, KC, 128], f32r, tag="xk", bufs=3,
                                    name=f"xk{t}")
                    xv_sb = p1.tile([128, KC, 128], f32r, tag="xv", bufs=3,
                                    name=f"xv{t}")
                    nc.sync.dma_start(out=xk_sb[:], in_=xk_t[t, :, :, :].bitcast(f32r))
                    nc.sync.dma_start(out=xv_sb[:], in_=xv_t[t, :, :, :].bitcast(f32r))
                    k_sb = p1.tile([128, E], f32r, tag="k", bufs=2, name=f"k{t}")
                    v_sb = p1.tile([128, E], f32r, tag="v", bufs=2, name=f"v{t}")
                    for hh in range(2):
                        psk = pp1.tile([128, 384], f32, tag="pp", bufs=2,
                                       name=f"psk{t}_{hh}")
                        # preload bias into psum, then accumulate on top
                        nc.scalar.activation(
                            psk[:], bkb_sb[:, hh * 384:(hh + 1) * 384], AF.Copy)
                        for kb in range(KC):
                            nc.tensor.matmul(
                                psk[:], xk_sb[:, kb, :],
                                wk_sb[:, kb, hh * 384:(hh + 1) * 384],
                                start=False, stop=(kb == KC - 1),
                                skip_group_check=True)
                        nc.scalar.activation(
                            k_sb[:, hh * 384:(hh + 1) * 384], psk[:], AF.Relu)
                    for hh in range(2):
                        psv = pp1.tile([128, 384], f32, tag="pp", bufs=2,
                                       name=f"psv{t}_{hh}")
                        nc.scalar.activation(
                            psv[:], bvb_sb[:, hh * 384:(hh + 1) * 384], AF.Copy)
                        for kb in range(KC):
                            nc.tensor.matmul(
                                psv[:], xv_sb[:, kb, :],
                                wv_sb[:, kb, hh * 384:(hh + 1) * 384],
                                start=False, stop=(kb == KC - 1),
                                skip_group_check=True)
                        nc.vector.tensor_copy(
                            v_sb[:, hh * 384:(hh + 1) * 384], psv[:])
                    for c in range(KC):
                        nc.tensor.matmul(
                            kvp[c][:, :],
                            k_sb[:, c * 128:(c + 1) * 128],
                            v_sb[:, c * 128:(c + 1) * 128],
                            start=(t == 0), stop=(t == NT - 1))

                # extract block-diagonal KV pairs into zeroed kvc
                nc.vector.memset(kvc[:], 0.0)
                for c in range(KC):
                    nc.scalar.activation(
                        kvc[0:64, c, 0:64],
                        kvp[c][0:64, 0:64], AF.Copy)
                    nc.scalar.activation(
                        kvc[64:128, c, 64:128],
                        kvp[c][64:128, 64:128], AF.Copy)
                nc.gpsimd.dma_start(kv_in[:], kvc[:])

            # ---------------- AllReduce KV over S-halves --------------------
            nc.gpsimd.collective_compute(
                "AllReduce",
                mybir.AluOpType.add,
                replica_groups=[[0, 1], [2, 3], [4, 5], [6, 7]],
                ins=[kv_in.opt()],
                outs=[kv_out.opt()],
            )
            nc.sync.dma_start(kvbd[:], kv_out[:].bitcast(f32r))

            # ---------------- phase 2/3: Q proj, QKV, out-proj --------------
            with (
                tc.tile_pool(name="p2", bufs=1) as p2,
                tc.tile_pool(name="pp2", bufs=1, space="PSUM") as pp2,
            ):
                def qproj(n):
                    xq_sb = xqs.pop(n)
                    qt = p2.tile([128, KC, 512], f32r, tag="qt", bufs=6,
                                 name=f"qt{n}")
                    for m in range(KC):
                        psq = pp2.tile([128, 512], f32, tag="pq", bufs=2,
                                       name=f"psq{n}_{m}")
                        for kb in range(KC):
                            nc.tensor.matmul(
                                psq[:], wq_sb[:, kb, m * 128:(m + 1) * 128],
                                xq_sb[:, kb, :],
                                start=(kb == 0), stop=(kb == KC - 1))
                        nc.scalar.activation(qt[:, m, :], psq[:], AF.Relu,
                                             bias=bqc_sb[:, m:m + 1])
                    return qt

                load_xq(3)
                load_xq(4)
                # run 5 windows of Q-proj ahead of the first QKV so the PE
                # stays busy while the KV AllReduce is in flight
                qts = {n: qproj(n) for n in range(5)}
                for n in range(NW):
                    if n + 5 < NW:
                        load_xq(n + 5)
                        qts[n + 5] = qproj(n + 5)
                    qt = qts.pop(n)
                    qkvt = p2.tile([128, KC, 512], f32r, tag="qkvt", bufs=2,
                                   name=f"qkvt{n}")
                    for c in range(KC):
                        psa = pp2.tile([128, 512], f32, tag="pa", bufs=2,
                                       name=f"psa{n}_{c}")
                        nc.tensor.matmul(psa[:], kvbd[:, c, :], qt[:, c, :],
                                         start=True, stop=True)
                        nc.vector.tensor_copy(qkvt[:, c, :], psa[:])
                    for m in range(KC):
                        pso = pp2.tile([128, 512], f32, tag="po", bufs=2,
                                       name=f"pso{n}_{m}")
                        for c in range(KC):
                            nc.tensor.matmul(
                                pso[:],
                                wo_sb[:, c, m * 128:(m + 1) * 128],
                                qkvt[:, c, :],
                                start=(c == 0), stop=(c == KC - 1))
                        yt_sb = p2.tile([128, 512], f32, tag="yt", bufs=3,
                                        name=f"yt{n}_{m}")
                        nc.scalar.activation(yt_sb[:], pso[:],
                                             AF.Identity,
                                             bias=boc_sb[:, m:m + 1])
                        nc.sync.dma_start(out=y_t[n, :, m, :], in_=yt_sb[:])
    return nc


def _prep_x_tiles(x, tile_free):
    # x: [SC, E] -> [SC//tile_free, 128, KC, tile_free]
    n = SC // tile_free
    return np.ascontiguousarray(
        x.T.reshape(KC, 128, n, tile_free).transpose(2, 1, 0, 3))


def _prep_w(W):
    # W: [E, E] (out, in) -> [128, KC, E] with [p, kb, eo] = W[eo, kb*128+p]
    return np.ascontiguousarray(W.T.reshape(KC, 128, E).transpose(1, 0, 2))


def _make_in_maps(query, key, value, Wq, bq, Wk, bk, Wv, bv, Wo, bo):
    query = np.asarray(query, dtype=np.float32)
    key = np.asarray(key, dtype=np.float32)
    value = np.asarray(value, dtype=np.float32)

    shared = {
        "wq_h": _prep_w(np.asarray(Wq, dtype=np.float32)),
        "wk_h": _prep_w(np.asarray(Wk, dtype=np.float32)),
        "wv_h": _prep_w(np.asarray(Wv, dtype=np.float32)),
        "wo_h": _prep_w(np.asarray(Wo, dtype=np.float32)),
        "bkb_h": np.ascontiguousarray(
            np.broadcast_to(np.asarray(bk, dtype=np.float32), (128, E))),
        "bvb_h": np.ascontiguousarray(
            np.broadcast_to(np.asarray(bv, dtype=np.float32), (128, E))),
        "bqc_h": np.ascontiguousarray(
            np.asarray(bq, dtype=np.float32).reshape(KC, 128).T),
        "boc_h": np.ascontiguousarray(
            np.asarray(bo, dtype=np.float32).reshape(KC, 128).T),
    }

    in_maps = []
    for i in range(N_CORES):
        b, hf = i // 2, i % 2
        sl = slice(hf * SC, (hf + 1) * SC)
        m = dict(shared)
        m["xq_t"] = _prep_x_tiles(query[b, sl], 512)
        m["xk_t"] = _prep_x_tiles(key[b, sl], 128)
        m["xv_t"] = _prep_x_tiles(value[b, sl], 128)
        in_maps.append(m)
    return in_maps


def kernel(**inputs):
    in_maps = _make_in_maps(**inputs)
    nc = _build_program()
    res = run_bass_kernel_spmd(nc, in_maps, core_ids=list(range(N_CORES)))

    out = np.empty((B, S, E), dtype=np.float32)
    for i in range(N_CORES):
        b, hf = i // 2, i % 2
        yt = res.results[i]["y_t"]  # [NW, 128, KC, 512]
        out[b, hf * SC:(hf + 1) * SC] = (
            yt.transpose(0, 3, 2, 1).reshape(SC, E))
    return out
